# revision 2
# baseline (speedup 1.0000x reference)
"""MoE layer (8 experts, top-2) on 8 Trainium2 NeuronCores — expert parallel.

Strategy
--------
Host (inside kernel(), cheap O(T*D) work):
  * gate: logits = x @ Wg, softmax, top-2, normalized combine weights
  * dispatch: gather each expert's tokens, pad to a common capacity CAP,
    pre-permute every operand into the exact on-chip layout so each DMA
    moves long contiguous runs per partition
  * combine: out[t] += w * (y + b2[e]) scatter-add

Device (one expert per core, SPMD over 8 cores, >99% of FLOPs):
  * yT = W2[e].T @ gelu(W1[e].T @ xT + b1[e])  with all matmuls on TensorE
  * weights resident in SBUF (bf16), fp32 PSUM accumulation
  * layout keeps D/F on partitions and tokens on the matmul free dim, so
    no transposes are needed anywhere on device
  * token dim split into near-equal tiles (<=512); per-tile matmul count
    is fixed (512), so balanced tiles beat a small remainder tile

NEFF fixed overhead scales with the number of dma_start instructions
(~0.4us each in the head/tail semaphore storms), so DMAs are batched:
W1 in a few graded pieces (small first so compute starts early), W2 in
4 pieces, x in 2, y one per token tile. All DRAM layouts match SBUF
exactly -> maximal contiguous runs per partition.

Returns the full [B, S, D] float32 output.
"""

import os
import sys

for _p in ("/opt/trn_rl_repo",):
    if _p not in sys.path:
        sys.path.insert(0, _p)

import numpy as np
import ml_dtypes

import concourse.bass as bass
import concourse.mybir as mybir
import concourse.tile as tile
from concourse import bacc
from concourse.bass_utils import run_bass_kernel_spmd

D_MODEL = 1024
D_FF = 4096
NUM_EXPERTS = 8
TOP_K = 2
N_CORES = 8
P = 128  # SBUF partitions

DC = D_MODEL // P   # 8 chunks of the model dim
FC = D_FF // P      # 32 chunks of the ffn dim

# W1 piece sizes in fc-chunks (graded: small first so mm1 starts early,
# larger later once the consumption deadline has slack)
W1_PIECES = [2, 2, 3, 3, 3, 4, 4, 4, 4, 3]
assert sum(W1_PIECES) == FC
# W2 piece sizes in fc-chunks (ACT ring; deadline is mm2 of tile 0)
W2_PIECES = [8, 8, 8, 8]
assert sum(W2_PIECES) == FC

N_WARMUP = 18

LAST_EXEC_NS = None


def _install_profile_hook():
    """Provide antenv.axon_hooks (NTFF profiling) if the image lacks it."""
    import types
    import contextlib
    import ctypes
    try:
        from antenv.axon_hooks import get_axon_ntff_profile_hook  # noqa: F401
        return
    except ImportError:
        pass
    so = "/opt/axon/libaxon_pjrt.so"
    if not os.path.exists(so):
        return
    lib = ctypes.CDLL(so)
    if not hasattr(lib, "axon_start_nrt_profile"):
        return
    lib.axon_start_nrt_profile.argtypes = [ctypes.POINTER(ctypes.c_int64),
                                           ctypes.c_size_t]
    lib.axon_start_nrt_profile.restype = ctypes.c_int64
    lib.axon_stop_nrt_profile.argtypes = [ctypes.c_char_p]
    lib.axon_stop_nrt_profile.restype = ctypes.c_int64

    @contextlib.contextmanager
    def _hook(output_dir, device_ids):
        import jax
        jax.devices()
        if device_ids:
            ids = (ctypes.c_int64 * len(device_ids))(*device_ids)
            rc = lib.axon_start_nrt_profile(ids, len(device_ids))
        else:
            rc = lib.axon_start_nrt_profile(None, 0)
        try:
            yield
        finally:
            if rc == 0:
                n = lib.axon_stop_nrt_profile(str(output_dir).encode())
                print(f"profile: {n} ntff file(s) -> {output_dir}",
                      file=sys.stderr)

    mod = types.ModuleType("antenv.axon_hooks")
    mod.get_axon_ntff_profile_hook = lambda: _hook
    mod.set_axon_ntff_profile_hook = lambda h: None
    sys.modules["antenv.axon_hooks"] = mod
    import antenv
    antenv.axon_hooks = mod
    import concourse.bass_utils as _bu
    _bu.upload_artifacts = lambda tmpdir: tmpdir


def _tile_shape(max_cnt):
    """Equal even tile size (<=512) and count covering max_cnt tokens."""
    lo = max(256, max_cnt)
    n = (lo + 511) // 512
    tn = -(-lo // n)
    tn += tn % 2
    return tn, n


def _build_program(tn, ntiles):
    """SPMD program: one expert's FFN over ntiles*tn tokens, bf16 matmuls.

    Two HWDGE rings (each FIFO): SP carries W1 pieces then y out; ACT
    carries x/b1 in then W2 pieces. DRAM layouts match SBUF exactly:
      xT0  [P, DC, tn]          xT0[p, dc, t]    = x[t, dc*128+p]
      xTr  [P, DC, (nt-1)*tn]   rest of the tokens, same layout
      W1   [P, FC, DC, 128]     W1[p, fc, dc, j] = W1[dc*128+p, fc*128+j]
      W2   [P, FC, D]           W2[p, fc, d]     = W2[fc*128+p, d]
      b1   [P, FC]              b1[p, fc]        = b1[fc*128+p]
    Output yT [D, ntiles*tn] float32 (= y.T).
    """
    cap = tn * ntiles
    bf16 = mybir.dt.bfloat16
    f32 = mybir.dt.float32
    nc = bacc.Bacc("TRN2", target_bir_lowering=False, debug=False,
                   num_devices=N_CORES)

    xT0_d = nc.dram_tensor("xT0", [P, DC, tn], bf16, kind="ExternalInput").ap()
    if ntiles > 1:
        xTr_d = nc.dram_tensor("xTr", [P, DC, (ntiles - 1) * tn], bf16,
                               kind="ExternalInput").ap()
    w1_d = nc.dram_tensor("W1", [P, FC, DC, P], bf16, kind="ExternalInput").ap()
    w2_d = nc.dram_tensor("W2", [P, FC, D_MODEL], bf16,
                          kind="ExternalInput").ap()
    b1_d = nc.dram_tensor("b1", [P, FC], f32, kind="ExternalInput").ap()
    yT_d = nc.dram_tensor("yT", [D_MODEL, cap], f32, kind="ExternalOutput").ap()

    with tile.TileContext(nc) as tc:
        with (
            tc.tile_pool(name="wpool", bufs=1) as wpool,
            tc.tile_pool(name="hpool", bufs=1) as hpool,
            tc.tile_pool(name="ypool", bufs=2) as ypool,
            tc.tile_pool(name="ph", bufs=3, space="PSUM") as ph_pool,
            tc.tile_pool(name="py", bufs=3, space="PSUM") as py_pool,
        ):
            # ACT ring: x tile 0, bias, rest of x, then W2 pieces
            xs0 = wpool.tile([P, DC, tn], bf16, tag="xs0", name="xs0")
            nc.scalar.dma_start(xs0[:], xT0_d)
            b1s = wpool.tile([P, FC], f32)
            nc.scalar.dma_start(b1s[:], b1_d)
            if ntiles > 1:
                xsr = wpool.tile([P, DC, (ntiles - 1) * tn], bf16, tag="xsr",
                                 name="xsr")
                nc.scalar.dma_start(xsr[:], xTr_d)
            w2q = []
            c0 = 0
            for i, nch in enumerate(W2_PIECES):
                wq = wpool.tile([P, nch, D_MODEL], bf16, tag=f"w2q{i}",
                                name=f"w2q{i}")
                nc.scalar.dma_start(wq[:], w2_d[:, c0:c0 + nch, :])
                w2q.append((c0, wq))
                c0 += nch

            def x_slice(ti, dc):
                if ti == 0:
                    return xs0[:, dc, :]
                t0 = (ti - 1) * tn
                return xsr[:, dc, t0:t0 + tn]

            def w2_slice(fc, dc):
                for c0, wq in reversed(w2q):
                    if fc >= c0:
                        return wq[:, fc - c0, dc * P:(dc + 1) * P]

            # SP ring: W1 pieces in consumption order (y DMAs ride after)
            w1q = []
            c0 = 0
            for i, nch in enumerate(W1_PIECES):
                wq = wpool.tile([P, nch, DC, P], bf16, tag=f"w1q{i}",
                                name=f"w1q{i}")
                nc.sync.dma_start(wq[:], w1_d[:, c0:c0 + nch, :, :])
                w1q.append((c0, wq))
                c0 += nch

            def w1_slice(fc, dc):
                for c0, wq in reversed(w1q):
                    if fc >= c0:
                        return wq[:, fc - c0, dc, :]

            # PE warm-up: dummy matmuls on scratch data while weights load,
            # so HAM un-throttles before the first real matmul
            warm = wpool.tile([P, 256], bf16)
            nc.vector.memset(warm[:], 0.0)
            wps, _ = tc.tile([P, 256], f32, space="PSUM", name="warmps")
            for _ in range(N_WARMUP):
                nc.tensor.matmul(wps[:], warm[:, :P], warm[:], start=True,
                                 stop=True)

            for ti in range(ntiles):
                t0 = ti * tn
                # hT = gelu(W1.T @ x + b1), layout [F(part), tokens]
                hT = hpool.tile([P, FC, tn], bf16, tag="hT")
                for fc in range(FC):
                    ph = ph_pool.tile([P, tn], f32, tag="ph")
                    for dc in range(DC):
                        nc.tensor.matmul(
                            ph[:],
                            w1_slice(fc, dc),
                            x_slice(ti, dc),
                            start=(dc == 0),
                            stop=(dc == DC - 1),
                        )
                    nc.scalar.activation(
                        hT[:, fc, :], ph[:],
                        mybir.ActivationFunctionType.Gelu,
                        bias=b1s[:, fc:fc + 1], scale=1.0,
                    )

                # yT = W2.T @ hT, layout [D(part), tokens]
                yt = ypool.tile([P, DC, tn], f32, tag="yt")
                for dc in range(DC):
                    py = py_pool.tile([P, tn], f32, tag="py")
                    for fc in range(FC):
                        nc.tensor.matmul(
                            py[:],
                            w2_slice(fc, dc),
                            hT[:, fc, :],
                            start=(fc == 0),
                            stop=(fc == FC - 1),
                        )
                    nc.vector.tensor_copy(yt[:, dc, :], py[:])

                # one output DMA per tile (last tile split so the drain
                # after the final matmul is short)
                dst = yT_d.rearrange("(dc p) t -> p dc t", p=P)
                if ti < ntiles - 1:
                    nc.sync.dma_start(dst[:, :, t0:t0 + tn], yt[:])
                else:
                    nc.sync.dma_start(dst[:, :DC // 2, t0:t0 + tn],
                                      yt[:, :DC // 2, :])
                    nc.sync.dma_start(dst[:, DC // 2:3 * DC // 4, t0:t0 + tn],
                                      yt[:, DC // 2:3 * DC // 4, :])
                    nc.sync.dma_start(dst[:, 3 * DC // 4:, t0:t0 + tn],
                                      yt[:, 3 * DC // 4:, :])

    nc.compile()
    return nc


def _route(x_flat, Wg):
    """Replicate the reference gate in float64: softmax, top-2, renorm."""
    logits = x_flat.astype(np.float64) @ Wg.astype(np.float64)
    logits -= logits.max(axis=-1, keepdims=True)
    p = np.exp(logits)
    p /= p.sum(axis=-1, keepdims=True)
    order = np.argsort(-p, axis=-1, kind="stable")[:, :TOP_K]   # [T, 2]
    rows = np.arange(p.shape[0])[:, None]
    tv = p[rows, order]                                          # [T, 2]
    tvn = tv / (tv.sum(axis=-1, keepdims=True) + 1e-8)
    return order, tvn


def kernel(x, Wg, W1, b1, W2, b2):
    global LAST_EXEC_NS
    x = np.asarray(x, dtype=np.float32)
    Wg = np.asarray(Wg, dtype=np.float32)
    W1 = np.asarray(W1, dtype=np.float32)
    b1 = np.asarray(b1, dtype=np.float32)
    W2 = np.asarray(W2, dtype=np.float32)
    b2 = np.asarray(b2, dtype=np.float32)

    B, S, D = x.shape
    x_flat = x.reshape(-1, D)
    T = x_flat.shape[0]

    order, tvn = _route(x_flat, Wg)

    idx = []
    wts = []
    for e in range(NUM_EXPERTS):
        sel = np.nonzero((order == e).any(axis=1))[0]
        idx.append(sel)
        wmat = np.where(order[sel] == e, tvn[sel], 0.0)
        wts.append(wmat.sum(axis=-1))                            # [cnt]

    max_cnt = max(len(s) for s in idx)
    tn, ntiles = _tile_shape(max_cnt)
    cap = tn * ntiles

    # a Bass program object must not be re-run after lowering (re-executing
    # a reused module corrupted the device) — build fresh every call; the
    # neuron compile cache keeps repeat builds fast
    nc = _build_program(tn, ntiles)

    bf16 = ml_dtypes.bfloat16
    in_maps = []
    for e in range(NUM_EXPERTS):
        sel = idx[e]
        xT = np.zeros((P, DC, cap), dtype=bf16)
        # [cnt, D] -> [cnt, DC, P] -> [P, DC, cnt]
        xT[:, :, :len(sel)] = x_flat[sel].reshape(-1, DC, P).transpose(2, 1, 0)
        # [D, F] -> [DC, P, FC, 128] -> [P, FC, DC, 128]
        w1e = np.ascontiguousarray(
            W1[e].reshape(DC, P, FC, P).transpose(1, 2, 0, 3)).astype(bf16)
        # [F, D] -> [FC, P, D] -> [P, FC, D]
        w2e = np.ascontiguousarray(
            W2[e].reshape(FC, P, D_MODEL).transpose(1, 0, 2)).astype(bf16)
        im = {
            "xT0": np.ascontiguousarray(xT[:, :, :tn]),
            "W1": w1e,
            "W2": w2e,
            "b1": np.ascontiguousarray(b1[e].reshape(FC, P).T),
        }
        if ntiles > 1:
            im["xTr"] = np.ascontiguousarray(xT[:, :, tn:])
        in_maps.append(im)

    trace = bool(os.environ.get("MOE_TRACE"))
    _install_profile_hook()   # also covers a harness-set BASS_TRACE=1
    try:
        res = run_bass_kernel_spmd(
            nc, in_maps, list(range(N_CORES)),
            trace=trace,
            tmpdir=os.environ.get("MOE_TRACE_DIR") or None,
        )
    except Exception:
        if not (trace or os.environ.get("BASS_TRACE")):
            raise
        # profiling path failed (e.g. no NTFF support) — run without it
        os.environ["BASS_NEVER_TRACE"] = "1"
        res = run_bass_kernel_spmd(nc, in_maps, list(range(N_CORES)))
    LAST_EXEC_NS = res.exec_time_ns

    out = np.zeros((T, D_MODEL), dtype=np.float64)
    for e in range(NUM_EXPERTS):
        sel = idx[e]
        yT = np.asarray(res.results[e]["yT"])                    # [D, cap] f32
        y = yT[:, :len(sel)].T.astype(np.float64)
        out[sel] += wts[e][:, None] * (y + b2[e].astype(np.float64))

    return out.reshape(B, S, D_MODEL).astype(np.float32)


# revision 8
# speedup vs baseline: 1.0657x; 1.0657x over previous
"""MoE layer (8 experts, top-2) on 8 Trainium2 NeuronCores — expert parallel.

Strategy
--------
Host (inside kernel(), cheap O(T*D) work):
  * gate: logits = x @ Wg, softmax, top-2, normalized combine weights
  * dispatch: gather each expert's tokens, pad to a common capacity CAP,
    pre-permute every operand into the exact on-chip layout so each DMA
    moves long contiguous runs per partition
  * combine: out[t] += w * (y + b2[e]) scatter-add

Device (one expert per core, SPMD over 8 cores, >99% of FLOPs):
  * yT = W2[e].T @ gelu(W1[e].T @ xT + b1[e])  with all matmuls on TensorE
  * weights resident in SBUF (bf16), fp32 PSUM accumulation
  * layout keeps D/F on partitions and tokens on the matmul free dim, so
    no transposes are needed anywhere on device
  * token dim split into near-equal tiles (<=512); per-tile matmul count
    is fixed (512), so balanced tiles beat a small remainder tile

NEFF fixed overhead scales with the number of dma_start instructions
(~0.4us each in the head/tail semaphore storms), so DMAs are batched:
W1 in a few graded pieces (small first so compute starts early), W2 in
4 pieces, x in 2, y one per token tile. All DRAM layouts match SBUF
exactly -> maximal contiguous runs per partition.

Returns the full [B, S, D] float32 output.
"""

import os
import sys

for _p in ("/opt/trn_rl_repo",):
    if _p not in sys.path:
        sys.path.insert(0, _p)

import numpy as np
import ml_dtypes

import concourse.bass as bass
import concourse.mybir as mybir
import concourse.tile as tile
from concourse import bacc
from concourse.bass_utils import run_bass_kernel_spmd

D_MODEL = 1024
D_FF = 4096
NUM_EXPERTS = 8
TOP_K = 2
N_CORES = 8
P = 128  # SBUF partitions

DC = D_MODEL // P   # 8 chunks of the model dim
FC = D_FF // P      # 32 chunks of the ffn dim

# W1 piece sizes in fc-chunks (graded: small first so mm1 starts early,
# larger later once the consumption deadline has slack). All weights ride
# the SP ring (~300 GB/s measured) in consumption order; mm2 is split in
# two dc-halves with fc outer so W2's deadline is spread over the whole
# first half-pass instead of a 5us burst the ring cannot feed.
W1_PIECES = [2, 2, 3, 3, 4, 5, 6, 7]
assert sum(W1_PIECES) == FC
W2_PIECES = [8, 6, 6, 6, 6]
assert sum(W2_PIECES) == FC

N_WARMUP = 18

LAST_EXEC_NS = None


def _install_profile_hook():
    """Provide antenv.axon_hooks (NTFF profiling) if the image lacks it."""
    import types
    import contextlib
    import ctypes
    try:
        from antenv.axon_hooks import get_axon_ntff_profile_hook  # noqa: F401
        return
    except ImportError:
        pass
    so = "/opt/axon/libaxon_pjrt.so"
    if not os.path.exists(so):
        return
    lib = ctypes.CDLL(so)
    if not hasattr(lib, "axon_start_nrt_profile"):
        return
    lib.axon_start_nrt_profile.argtypes = [ctypes.POINTER(ctypes.c_int64),
                                           ctypes.c_size_t]
    lib.axon_start_nrt_profile.restype = ctypes.c_int64
    lib.axon_stop_nrt_profile.argtypes = [ctypes.c_char_p]
    lib.axon_stop_nrt_profile.restype = ctypes.c_int64

    @contextlib.contextmanager
    def _hook(output_dir, device_ids):
        import jax
        jax.devices()
        if device_ids:
            ids = (ctypes.c_int64 * len(device_ids))(*device_ids)
            rc = lib.axon_start_nrt_profile(ids, len(device_ids))
        else:
            rc = lib.axon_start_nrt_profile(None, 0)
        try:
            yield
        finally:
            if rc == 0:
                n = lib.axon_stop_nrt_profile(str(output_dir).encode())
                print(f"profile: {n} ntff file(s) -> {output_dir}",
                      file=sys.stderr)

    mod = types.ModuleType("antenv.axon_hooks")
    mod.get_axon_ntff_profile_hook = lambda: _hook
    mod.set_axon_ntff_profile_hook = lambda h: None
    sys.modules["antenv.axon_hooks"] = mod
    import antenv
    antenv.axon_hooks = mod
    import concourse.bass_utils as _bu
    _bu.upload_artifacts = lambda tmpdir: tmpdir


def _tile_shape(max_cnt):
    """Equal even tile size (<=512) and count covering max_cnt tokens."""
    lo = max(256, max_cnt)
    n = (lo + 511) // 512
    tn = -(-lo // n)
    tn += tn % 2
    return tn, n


def _build_program(tn, ntiles):
    """SPMD program: one expert's FFN over ntiles*tn tokens, bf16 matmuls.

    Two HWDGE rings (each FIFO): SP carries W1 pieces then y out; ACT
    carries x/b1 in then W2 pieces. DRAM layouts match SBUF exactly:
      xT0  [P, DC, tn]          xT0[p, dc, t]    = x[t, dc*128+p]
      xTr  [P, DC, (nt-1)*tn]   rest of the tokens, same layout
      W1   [P, FC, DC, 128]     W1[p, fc, dc, j] = W1[dc*128+p, fc*128+j]
      W2   [P, FC, D]           W2[p, fc, d]     = W2[fc*128+p, d]
      b1   [P, FC]              b1[p, fc]        = b1[fc*128+p]
    Output yT [D, ntiles*tn] float32 (= y.T).
    """
    cap = tn * ntiles
    bf16 = mybir.dt.bfloat16
    f32 = mybir.dt.float32
    nc = bacc.Bacc("TRN2", target_bir_lowering=False, debug=False,
                   num_devices=N_CORES)

    xT0_d = nc.dram_tensor("xT0", [P, DC, tn], bf16, kind="ExternalInput").ap()
    if ntiles > 1:
        xTr_d = nc.dram_tensor("xTr", [P, DC, (ntiles - 1) * tn], bf16,
                               kind="ExternalInput").ap()
    w1_d = nc.dram_tensor("W1", [P, FC, DC, P], bf16, kind="ExternalInput").ap()
    w2_d = nc.dram_tensor("W2", [P, FC, D_MODEL], bf16,
                          kind="ExternalInput").ap()
    b1_d = nc.dram_tensor("b1", [P, FC], f32, kind="ExternalInput").ap()
    yT_d = nc.dram_tensor("yT", [D_MODEL, cap], f32, kind="ExternalOutput").ap()

    with tile.TileContext(nc) as tc:
        with (
            tc.tile_pool(name="wpool", bufs=1) as wpool,
            tc.tile_pool(name="hpool", bufs=1) as hpool,
            tc.tile_pool(name="ypool", bufs=2) as ypool,
            tc.tile_pool(name="ph", bufs=2, space="PSUM") as ph_pool,
            tc.tile_pool(name="py", bufs=1, space="PSUM") as py_pool,
        ):
            # ACT ring: x tile 0, bias, rest of x (y output rides later)
            xs0 = wpool.tile([P, DC, tn], bf16, tag="xs0", name="xs0")
            nc.scalar.dma_start(xs0[:], xT0_d)
            b1s = wpool.tile([P, FC], f32)
            nc.scalar.dma_start(b1s[:], b1_d)
            if ntiles > 1:
                xsr = wpool.tile([P, DC, (ntiles - 1) * tn], bf16, tag="xsr",
                                 name="xsr")
                nc.scalar.dma_start(xsr[:], xTr_d)

            def x_slice(ti, dc):
                if ti == 0:
                    return xs0[:, dc, :]
                t0 = (ti - 1) * tn
                return xsr[:, dc, t0:t0 + tn]

            # SP ring: W1 pieces then W2 pieces, in consumption order
            w1q = []
            c0 = 0
            for i, nch in enumerate(W1_PIECES):
                wq = wpool.tile([P, nch, DC, P], bf16, tag=f"w1q{i}",
                                name=f"w1q{i}")
                nc.sync.dma_start(wq[:], w1_d[:, c0:c0 + nch, :, :])
                w1q.append((c0, wq))
                c0 += nch

            def w1_slice(fc, dc):
                for c0, wq in reversed(w1q):
                    if fc >= c0:
                        return wq[:, fc - c0, dc, :]

            w2q = []
            c0 = 0
            for i, nch in enumerate(W2_PIECES):
                wq = wpool.tile([P, nch, D_MODEL], bf16, tag=f"w2q{i}",
                                name=f"w2q{i}")
                nc.sync.dma_start(wq[:], w2_d[:, c0:c0 + nch, :])
                w2q.append((c0, wq))
                c0 += nch

            def w2_slice(fc, dc):
                for c0, wq in reversed(w2q):
                    if fc >= c0:
                        return wq[:, fc - c0, dc * P:(dc + 1) * P]

            # PE warm-up: dummy matmuls on scratch data while weights load,
            # so HAM un-throttles before the first real matmul
            warm = wpool.tile([P, 256], bf16)
            nc.vector.memset(warm[:], 0.0)
            wps, _ = tc.tile([P, 256], f32, space="PSUM", name="warmps")
            for _ in range(N_WARMUP):
                nc.tensor.matmul(wps[:], warm[:, :P], warm[:], start=True,
                                 stop=True)

            for ti in range(ntiles):
                t0 = ti * tn
                # hT = gelu(W1.T @ x + b1), layout [F(part), tokens]
                hT = hpool.tile([P, FC, tn], bf16, tag="hT")
                for fc in range(FC):
                    ph = ph_pool.tile([P, tn], f32, tag="ph")
                    for dc in range(DC):
                        nc.tensor.matmul(
                            ph[:],
                            w1_slice(fc, dc),
                            x_slice(ti, dc),
                            start=(dc == 0),
                            stop=(dc == DC - 1),
                        )
                    nc.scalar.activation(
                        hT[:, fc, :], ph[:],
                        mybir.ActivationFunctionType.Gelu,
                        bias=b1s[:, fc:fc + 1], scale=1.0,
                    )

                # yT = W2.T @ hT, layout [D(part), tokens]. fc is the OUTER
                # loop (dc-half inner) so W2 chunk fc is first needed ~fc
                # matmul-rounds into the pass — the weight ring can stream
                # W2 during the pass instead of needing it all upfront.
                dst = yT_d.rearrange("(dc p) t -> p dc t", p=P)
                yt = ypool.tile([P, DC, tn], f32, tag="yt")
                half = DC // 2
                for h in range(2):
                    gp = 2 * ti + h
                    dcs = range(h * half, (h + 1) * half)
                    # 5 psum tags round-robined 4-per-pass: the tag a pass
                    # reuses is the previous pass's FIRST-drained bank, so
                    # the start=True matmul never waits on a pending copy
                    pys = {dc: py_pool.tile([P, tn], f32,
                                            tag=f"py{(gp * 4 + k) % 5}",
                                            name=f"py_t{ti}h{h}d{dc}")
                           for k, dc in enumerate(dcs)}
                    for fc in range(FC):
                        for dc in dcs:
                            nc.tensor.matmul(
                                pys[dc][:],
                                w2_slice(fc, dc),
                                hT[:, fc, :],
                                start=(fc == 0),
                                stop=(fc == FC - 1),
                            )
                    for dc in dcs:
                        nc.vector.tensor_copy(yt[:, dc, :], pys[dc][:])
                    if ti < ntiles - 1:
                        if h == 1:
                            nc.scalar.dma_start(dst[:, :, t0:t0 + tn], yt[:])
                    else:
                        # last tile: drain output in pieces so little is
                        # left after the final matmul
                        if h == 0:
                            nc.scalar.dma_start(dst[:, :half, t0:t0 + tn],
                                                yt[:, :half, :])
                        else:
                            nc.scalar.dma_start(
                                dst[:, half:DC - 1, t0:t0 + tn],
                                yt[:, half:DC - 1, :])
                            nc.scalar.dma_start(dst[:, DC - 1:, t0:t0 + tn],
                                                yt[:, DC - 1:, :])

    nc.compile()
    return nc


def _route(x_flat, Wg):
    """Replicate the reference gate in float64: softmax, top-2, renorm."""
    logits = x_flat.astype(np.float64) @ Wg.astype(np.float64)
    logits -= logits.max(axis=-1, keepdims=True)
    p = np.exp(logits)
    p /= p.sum(axis=-1, keepdims=True)
    order = np.argsort(-p, axis=-1, kind="stable")[:, :TOP_K]   # [T, 2]
    rows = np.arange(p.shape[0])[:, None]
    tv = p[rows, order]                                          # [T, 2]
    tvn = tv / (tv.sum(axis=-1, keepdims=True) + 1e-8)
    return order, tvn


def kernel(x, Wg, W1, b1, W2, b2):
    global LAST_EXEC_NS
    x = np.asarray(x, dtype=np.float32)
    Wg = np.asarray(Wg, dtype=np.float32)
    W1 = np.asarray(W1, dtype=np.float32)
    b1 = np.asarray(b1, dtype=np.float32)
    W2 = np.asarray(W2, dtype=np.float32)
    b2 = np.asarray(b2, dtype=np.float32)

    B, S, D = x.shape
    x_flat = x.reshape(-1, D)
    T = x_flat.shape[0]

    order, tvn = _route(x_flat, Wg)

    idx = []
    wts = []
    for e in range(NUM_EXPERTS):
        sel = np.nonzero((order == e).any(axis=1))[0]
        idx.append(sel)
        wmat = np.where(order[sel] == e, tvn[sel], 0.0)
        wts.append(wmat.sum(axis=-1))                            # [cnt]

    max_cnt = max(len(s) for s in idx)
    tn, ntiles = _tile_shape(max_cnt)
    cap = tn * ntiles

    # a Bass program object must not be re-run after lowering (re-executing
    # a reused module corrupted the device) — build fresh every call; the
    # neuron compile cache keeps repeat builds fast
    nc = _build_program(tn, ntiles)

    bf16 = ml_dtypes.bfloat16
    in_maps = []
    for e in range(NUM_EXPERTS):
        sel = idx[e]
        xT = np.zeros((P, DC, cap), dtype=bf16)
        # [cnt, D] -> [cnt, DC, P] -> [P, DC, cnt]
        xT[:, :, :len(sel)] = x_flat[sel].reshape(-1, DC, P).transpose(2, 1, 0)
        # [D, F] -> [DC, P, FC, 128] -> [P, FC, DC, 128]
        w1e = np.ascontiguousarray(
            W1[e].reshape(DC, P, FC, P).transpose(1, 2, 0, 3)).astype(bf16)
        # [F, D] -> [FC, P, D] -> [P, FC, D]
        w2e = np.ascontiguousarray(
            W2[e].reshape(FC, P, D_MODEL).transpose(1, 0, 2)).astype(bf16)
        im = {
            "xT0": np.ascontiguousarray(xT[:, :, :tn]),
            "W1": w1e,
            "W2": w2e,
            "b1": np.ascontiguousarray(b1[e].reshape(FC, P).T),
        }
        if ntiles > 1:
            im["xTr"] = np.ascontiguousarray(xT[:, :, tn:])
        in_maps.append(im)

    trace = bool(os.environ.get("MOE_TRACE"))
    _install_profile_hook()   # also covers a harness-set BASS_TRACE=1
    try:
        res = run_bass_kernel_spmd(
            nc, in_maps, list(range(N_CORES)),
            trace=trace,
            tmpdir=os.environ.get("MOE_TRACE_DIR") or None,
        )
    except Exception:
        if not (trace or os.environ.get("BASS_TRACE")):
            raise
        # profiling path failed (e.g. no NTFF support) — run without it
        os.environ["BASS_NEVER_TRACE"] = "1"
        res = run_bass_kernel_spmd(nc, in_maps, list(range(N_CORES)))
    LAST_EXEC_NS = res.exec_time_ns

    out = np.zeros((T, D_MODEL), dtype=np.float64)
    for e in range(NUM_EXPERTS):
        sel = idx[e]
        yT = np.asarray(res.results[e]["yT"])                    # [D, cap] f32
        y = yT[:, :len(sel)].T.astype(np.float64)
        out[sel] += wts[e][:, None] * (y + b2[e].astype(np.float64))

    return out.reshape(B, S, D_MODEL).astype(np.float32)


# revision 13
# speedup vs baseline: 1.1151x; 1.0464x over previous
"""MoE layer (8 experts, top-2) on 8 Trainium2 NeuronCores — expert parallel.

Strategy
--------
Host (inside kernel(), cheap O(T*D) work):
  * gate: logits = x @ Wg, softmax, top-2, normalized combine weights
  * dispatch: gather each expert's tokens, pad to a common capacity CAP,
    pre-permute every operand into the exact on-chip layout so each DMA
    moves long contiguous runs per partition
  * combine: out[t] += w * (y + b2[e]) scatter-add

Device (one expert per core, SPMD over 8 cores, >99% of FLOPs):
  * yT = W2[e].T @ gelu(W1[e].T @ xT + b1[e])  with all matmuls on TensorE
  * weights resident in SBUF (bf16), fp32 PSUM accumulation
  * layout keeps D/F on partitions and tokens on the matmul free dim, so
    no transposes are needed anywhere on device
  * token dim split into near-equal tiles (<=512); per-tile matmul count
    is fixed (512), so balanced tiles beat a small remainder tile

NEFF fixed overhead scales with the number of dma_start instructions
(~0.4us each in the head/tail semaphore storms), so DMAs are batched:
W1 in a few graded pieces (small first so compute starts early), W2 in
4 pieces, x in 2, y one per token tile. All DRAM layouts match SBUF
exactly -> maximal contiguous runs per partition.

Returns the full [B, S, D] float32 output.
"""

import os
import sys

for _p in ("/opt/trn_rl_repo",):
    if _p not in sys.path:
        sys.path.insert(0, _p)

import numpy as np
import ml_dtypes

import concourse.bass as bass
import concourse.mybir as mybir
import concourse.tile as tile
from concourse import bacc
from concourse.bass_utils import run_bass_kernel_spmd

D_MODEL = 1024
D_FF = 4096
NUM_EXPERTS = 8
TOP_K = 2
N_CORES = 8
P = 128  # SBUF partitions

DC = D_MODEL // P   # 8 chunks of the model dim
FC = D_FF // P      # 32 chunks of the ffn dim

# W1 piece sizes in fc-chunks (graded: small first so mm1 starts early,
# larger later once the consumption deadline has slack). All weights ride
# the SP ring (~300 GB/s measured) in consumption order; mm2 is split in
# two dc-halves with fc outer so W2's deadline is spread over the whole
# first half-pass instead of a 5us burst the ring cannot feed.
W1_PIECES = [1, 1, 2, 3, 4, 5, 8, 8]
assert sum(W1_PIECES) == FC
W2_PIECES = [8, 6, 6, 6, 6]
assert sum(W2_PIECES) == FC

N_WARMUP = 18

LAST_EXEC_NS = None


def _install_profile_hook():
    """Provide antenv.axon_hooks (NTFF profiling) if the image lacks it."""
    import types
    import contextlib
    import ctypes
    try:
        from antenv.axon_hooks import get_axon_ntff_profile_hook  # noqa: F401
        return
    except ImportError:
        pass
    so = "/opt/axon/libaxon_pjrt.so"
    if not os.path.exists(so):
        return
    lib = ctypes.CDLL(so)
    if not hasattr(lib, "axon_start_nrt_profile"):
        return
    lib.axon_start_nrt_profile.argtypes = [ctypes.POINTER(ctypes.c_int64),
                                           ctypes.c_size_t]
    lib.axon_start_nrt_profile.restype = ctypes.c_int64
    lib.axon_stop_nrt_profile.argtypes = [ctypes.c_char_p]
    lib.axon_stop_nrt_profile.restype = ctypes.c_int64

    @contextlib.contextmanager
    def _hook(output_dir, device_ids):
        import jax
        jax.devices()
        if device_ids:
            ids = (ctypes.c_int64 * len(device_ids))(*device_ids)
            rc = lib.axon_start_nrt_profile(ids, len(device_ids))
        else:
            rc = lib.axon_start_nrt_profile(None, 0)
        try:
            yield
        finally:
            if rc == 0:
                n = lib.axon_stop_nrt_profile(str(output_dir).encode())
                print(f"profile: {n} ntff file(s) -> {output_dir}",
                      file=sys.stderr)

    mod = types.ModuleType("antenv.axon_hooks")
    mod.get_axon_ntff_profile_hook = lambda: _hook
    mod.set_axon_ntff_profile_hook = lambda h: None
    sys.modules["antenv.axon_hooks"] = mod
    import antenv
    antenv.axon_hooks = mod
    import concourse.bass_utils as _bu
    _bu.upload_artifacts = lambda tmpdir: tmpdir


def _tile_shape(max_cnt):
    """Equal even tile size (<=512) and count covering max_cnt tokens."""
    lo = max(256, max_cnt)
    n = (lo + 511) // 512
    tn = -(-lo // n)
    tn += tn % 2
    return tn, n


def _build_program(tn, ntiles):
    """SPMD program: one expert's FFN over ntiles*tn tokens, bf16 matmuls.

    Two HWDGE rings (each FIFO): SP carries W1 pieces then y out; ACT
    carries x/b1 in then W2 pieces. DRAM layouts match SBUF exactly:
      xT0  [P, DC, tn]          xT0[p, dc, t]    = x[t, dc*128+p]
      xTr  [P, DC, (nt-1)*tn]   rest of the tokens, same layout
      W1   [P, FC, DC, 128]     W1[p, fc, dc, j] = W1[dc*128+p, fc*128+j]
      W2   [P, FC, D]           W2[p, fc, d]     = W2[fc*128+p, d]
      b1   [P, FC]              b1[p, fc]        = b1[fc*128+p]
    Output yT [D, ntiles*tn] float32 (= y.T).
    """
    cap = tn * ntiles
    bf16 = mybir.dt.bfloat16
    f32 = mybir.dt.float32
    nc = bacc.Bacc("TRN2", target_bir_lowering=False, debug=False,
                   num_devices=N_CORES)

    xT0_d = nc.dram_tensor("xT0", [P, DC, tn], bf16, kind="ExternalInput").ap()
    if ntiles > 1:
        xTr_d = nc.dram_tensor("xTr", [P, DC, (ntiles - 1) * tn], bf16,
                               kind="ExternalInput").ap()
    w1_d = nc.dram_tensor("W1", [P, FC, DC, P], bf16, kind="ExternalInput").ap()
    w2_d = nc.dram_tensor("W2", [P, FC, D_MODEL], bf16,
                          kind="ExternalInput").ap()
    b1_d = nc.dram_tensor("b1", [P, FC], f32, kind="ExternalInput").ap()
    yT_d = nc.dram_tensor("yT", [D_MODEL, cap], f32, kind="ExternalOutput").ap()

    with tile.TileContext(nc) as tc:
        with (
            tc.tile_pool(name="wpool", bufs=1) as wpool,
            tc.tile_pool(name="hpool", bufs=1) as hpool,
            tc.tile_pool(name="ypool", bufs=2) as ypool,
            tc.tile_pool(name="ph", bufs=2, space="PSUM") as ph_pool,
            tc.tile_pool(name="py", bufs=1, space="PSUM") as py_pool,
        ):
            # ACT ring: x tile 0 in two halves (the first 4 dc-chunks gate
            # the very first matmul group), bias, rest of x (y rides later)
            xs0a = wpool.tile([P, DC // 2, tn], bf16, tag="xs0a", name="xs0a")
            nc.scalar.dma_start(xs0a[:], xT0_d[:, :DC // 2, :])
            xs0b = wpool.tile([P, DC // 2, tn], bf16, tag="xs0b", name="xs0b")
            nc.scalar.dma_start(xs0b[:], xT0_d[:, DC // 2:, :])
            b1s = wpool.tile([P, FC], f32)
            nc.scalar.dma_start(b1s[:], b1_d)
            if ntiles > 1:
                xsr = wpool.tile([P, DC, (ntiles - 1) * tn], bf16, tag="xsr",
                                 name="xsr")
                nc.scalar.dma_start(xsr[:], xTr_d)

            def x_slice(ti, dc):
                if ti == 0:
                    if dc < DC // 2:
                        return xs0a[:, dc, :]
                    return xs0b[:, dc - DC // 2, :]
                t0 = (ti - 1) * tn
                return xsr[:, dc, t0:t0 + tn]

            # SP ring: W1 pieces then W2 pieces, in consumption order
            w1q = []
            c0 = 0
            for i, nch in enumerate(W1_PIECES):
                wq = wpool.tile([P, nch, DC, P], bf16, tag=f"w1q{i}",
                                name=f"w1q{i}")
                nc.sync.dma_start(wq[:], w1_d[:, c0:c0 + nch, :, :])
                w1q.append((c0, wq))
                c0 += nch

            def w1_slice(fc, dc):
                for c0, wq in reversed(w1q):
                    if fc >= c0:
                        return wq[:, fc - c0, dc, :]

            w2q = []
            c0 = 0
            for i, nch in enumerate(W2_PIECES):
                wq = wpool.tile([P, nch, D_MODEL], bf16, tag=f"w2q{i}",
                                name=f"w2q{i}")
                nc.sync.dma_start(wq[:], w2_d[:, c0:c0 + nch, :])
                w2q.append((c0, wq))
                c0 += nch

            def w2_slice(fc, dc):
                for c0, wq in reversed(w2q):
                    if fc >= c0:
                        return wq[:, fc - c0, dc * P:(dc + 1) * P]

            # PE warm-up: dummy matmuls on scratch data while weights load,
            # so HAM un-throttles before the first real matmul
            warm = wpool.tile([P, 256], bf16)
            nc.vector.memset(warm[:], 0.0)
            wps, _ = tc.tile([P, 256], f32, space="PSUM", name="warmps")
            for _ in range(N_WARMUP):
                nc.tensor.matmul(wps[:], warm[:, :P], warm[:], start=True,
                                 stop=True)

            for ti in range(ntiles):
                t0 = ti * tn
                # hT = gelu(W1.T @ x + b1), layout [F(part), tokens]
                hT = hpool.tile([P, FC, tn], bf16, tag="hT")
                for fc in range(FC):
                    ph = ph_pool.tile([P, tn], f32, tag="ph")
                    for dc in range(DC):
                        nc.tensor.matmul(
                            ph[:],
                            w1_slice(fc, dc),
                            x_slice(ti, dc),
                            start=(dc == 0),
                            stop=(dc == DC - 1),
                        )
                    nc.scalar.activation(
                        hT[:, fc, :], ph[:],
                        mybir.ActivationFunctionType.Gelu,
                        bias=b1s[:, fc:fc + 1], scale=1.0,
                    )

                # yT = W2.T @ hT, layout [D(part), tokens]. fc is the OUTER
                # loop (dc-half inner) so W2 chunk fc is first needed ~fc
                # matmul-rounds into the pass — the weight ring can stream
                # W2 during the pass instead of needing it all upfront.
                dst = yT_d.rearrange("(dc p) t -> p dc t", p=P)
                yt = ypool.tile([P, DC, tn], f32, tag="yt")
                half = DC // 2
                for h in range(2):
                    gp = 2 * ti + h
                    dcs = range(h * half, (h + 1) * half)
                    # 5 psum tags round-robined 4-per-pass: the tag a pass
                    # reuses is the previous pass's FIRST-drained bank, so
                    # the start=True matmul never waits on a pending copy
                    pys = {dc: py_pool.tile([P, tn], f32,
                                            tag=f"py{(gp * 4 + k) % 5}",
                                            name=f"py_t{ti}h{h}d{dc}")
                           for k, dc in enumerate(dcs)}
                    for fc in range(FC):
                        for dc in dcs:
                            nc.tensor.matmul(
                                pys[dc][:],
                                w2_slice(fc, dc),
                                hT[:, fc, :],
                                start=(fc == 0),
                                stop=(fc == FC - 1),
                            )
                    if ti == ntiles - 1 and h == 1:
                        # final half-pass: copies split across two engines
                        # (parallel chains) + per-dc DMAs so the drain after
                        # the last matmul is short
                        for k, dc in enumerate(dcs):
                            if k % 2 == 0:
                                nc.vector.tensor_copy(yt[:, dc, :], pys[dc][:])
                            else:
                                nc.scalar.activation(
                                    yt[:, dc, :], pys[dc][:],
                                    mybir.ActivationFunctionType.Copy,
                                    scale=1.0)
                            nc.scalar.dma_start(dst[:, dc:dc + 1, t0:t0 + tn],
                                                yt[:, dc, :])
                        continue
                    for dc in dcs:
                        nc.vector.tensor_copy(yt[:, dc, :], pys[dc][:])
                    if ti < ntiles - 1:
                        if h == 1:
                            nc.scalar.dma_start(dst[:, :, t0:t0 + tn], yt[:])
                    else:
                        # last tile, first half: drain early so little is
                        # left after the final matmul
                        nc.scalar.dma_start(dst[:, :half, t0:t0 + tn],
                                            yt[:, :half, :])

    nc.compile()
    return nc


def _route(x_flat, Wg):
    """Replicate the reference gate in float64: softmax, top-2, renorm."""
    logits = x_flat.astype(np.float64) @ Wg.astype(np.float64)
    logits -= logits.max(axis=-1, keepdims=True)
    p = np.exp(logits)
    p /= p.sum(axis=-1, keepdims=True)
    order = np.argsort(-p, axis=-1, kind="stable")[:, :TOP_K]   # [T, 2]
    rows = np.arange(p.shape[0])[:, None]
    tv = p[rows, order]                                          # [T, 2]
    tvn = tv / (tv.sum(axis=-1, keepdims=True) + 1e-8)
    return order, tvn


def kernel(x, Wg, W1, b1, W2, b2):
    global LAST_EXEC_NS
    x = np.asarray(x, dtype=np.float32)
    Wg = np.asarray(Wg, dtype=np.float32)
    W1 = np.asarray(W1, dtype=np.float32)
    b1 = np.asarray(b1, dtype=np.float32)
    W2 = np.asarray(W2, dtype=np.float32)
    b2 = np.asarray(b2, dtype=np.float32)

    B, S, D = x.shape
    x_flat = x.reshape(-1, D)
    T = x_flat.shape[0]

    order, tvn = _route(x_flat, Wg)

    idx = []
    wts = []
    for e in range(NUM_EXPERTS):
        sel = np.nonzero((order == e).any(axis=1))[0]
        idx.append(sel)
        wmat = np.where(order[sel] == e, tvn[sel], 0.0)
        wts.append(wmat.sum(axis=-1))                            # [cnt]

    max_cnt = max(len(s) for s in idx)
    tn, ntiles = _tile_shape(max_cnt)
    cap = tn * ntiles

    # a Bass program object must not be re-run after lowering (re-executing
    # a reused module corrupted the device) — build fresh every call; the
    # neuron compile cache keeps repeat builds fast
    nc = _build_program(tn, ntiles)

    bf16 = ml_dtypes.bfloat16
    in_maps = []
    for e in range(NUM_EXPERTS):
        sel = idx[e]
        xT = np.zeros((P, DC, cap), dtype=bf16)
        # [cnt, D] -> [cnt, DC, P] -> [P, DC, cnt]
        xT[:, :, :len(sel)] = x_flat[sel].reshape(-1, DC, P).transpose(2, 1, 0)
        # [D, F] -> [DC, P, FC, 128] -> [P, FC, DC, 128]
        w1e = np.ascontiguousarray(
            W1[e].reshape(DC, P, FC, P).transpose(1, 2, 0, 3)).astype(bf16)
        # [F, D] -> [FC, P, D] -> [P, FC, D]
        w2e = np.ascontiguousarray(
            W2[e].reshape(FC, P, D_MODEL).transpose(1, 0, 2)).astype(bf16)
        im = {
            "xT0": np.ascontiguousarray(xT[:, :, :tn]),
            "W1": w1e,
            "W2": w2e,
            "b1": np.ascontiguousarray(b1[e].reshape(FC, P).T),
        }
        if ntiles > 1:
            im["xTr"] = np.ascontiguousarray(xT[:, :, tn:])
        in_maps.append(im)

    trace = bool(os.environ.get("MOE_TRACE"))
    _install_profile_hook()   # also covers a harness-set BASS_TRACE=1
    try:
        res = run_bass_kernel_spmd(
            nc, in_maps, list(range(N_CORES)),
            trace=trace,
            tmpdir=os.environ.get("MOE_TRACE_DIR") or None,
        )
    except Exception:
        if not (trace or os.environ.get("BASS_TRACE")):
            raise
        # profiling path failed (e.g. no NTFF support) — run without it
        os.environ["BASS_NEVER_TRACE"] = "1"
        res = run_bass_kernel_spmd(nc, in_maps, list(range(N_CORES)))
    LAST_EXEC_NS = res.exec_time_ns

    out = np.zeros((T, D_MODEL), dtype=np.float64)
    for e in range(NUM_EXPERTS):
        sel = idx[e]
        yT = np.asarray(res.results[e]["yT"])                    # [D, cap] f32
        y = yT[:, :len(sel)].T.astype(np.float64)
        out[sel] += wts[e][:, None] * (y + b2[e].astype(np.float64))

    return out.reshape(B, S, D_MODEL).astype(np.float32)
